# revision 39
# baseline (speedup 1.0000x reference)
"""Llama4TextExperts MoE grouped-GEMM kernel for 8 Trainium2 NeuronCores.

Expert-parallel: core e owns expert e and the pre-sorted token block
hidden_states[e*4096:(e+1)*4096]. No collectives needed.

The host pre-packs all operands into bf16 device layouts so the PE does
nothing but the 6144 GEMM matmuls (no on-chip transposes):
  xt [4, 128, 16, 1024]  xt[c,p,k,t] = x[c*1024+t, k*128+p]  (xT, chunked)
  w1 [32, 128, 2048]     w1[mp*2+gu] = W1 column block, k-tiled, contiguous
  w2 [128, 16, 2048]     w2[p,k2,h]  = W2[k2*128+p, h]

Per-core pipeline over 4 token chunks of TC=1024:
  mm1: for each of 16 gate/up column-block pairs, accumulate
       psg/psu [128,512] over k=16 (bf16 matmuls, f32 PSUM),
       ACT silu -> bf16, DVE mul -> actT bf16 [D-part, T free]
  mm2: actT block as stationary, W2 rows as moving -> natural [token, H]
       PSUM output; DVE copy -> bf16 SBUF, store DMA on the scalar queue.

Scheduling notes (all trace-verified):
- The PE stream runs at 216 ns per 512-wide matmul (512 rows at the
  2.37 GHz DVFS-capped clock; LDWEIGHTS hidden). The optimization
  budget is only the ~20 us outside the stream.
- Startup: the first load packet lands ~8.7 us (fixed preamble), and
  per-core HBM read bandwidth is ~0.42 MB/us SHARED across all DMA
  rings (splitting loads over sync+gpsimd was measured to give zero
  aggregate gain). A plain mp-at-a-time mm1 consumes x at 0.59 MB/us
  and stalls on the ring; chunk 0 therefore runs its first TWO mp
  iterations k-outer with gate+up for both mps accumulating across
  all 8 PSUM banks at once, dropping consumption to 0.145 MB/us —
  the stream runs stall-free, which also removes the DVFS-reset
  failure mode on slow-supply runs. W2's 8 MB is threaded in
  quarters into the mp=6..12 weight loads where the ring has slack.
- 10 warm-up matmuls on a zero tile ride the initial DMA wait and pull
  the PE out of its low p-state right as the real stream begins (the
  DVFS ramp needs ~3.6 us of busy PE; an idle gap > ~2 us resets it —
  the stream start is data-walled at ~13.5 us, so the extra warm-ups
  are reset-insurance, not delay).
- The startup stationaries ride a host-packed contiguous tensor:
  k-half slices of the regular w1 tiles make 2 KB-per-partition
  strided DMAs that run at HALF ring rate (measured +2.5 us gate).
- mm2 runs hq-outer so each [128,512] output group retires 3.5 us
  before the next: the final block's copies (split ACT/DVE) and
  stores (split scalar/sync rings) drain right behind the last matmul
  (tail measured 5.2 us vs 7.5 us for tb-batched stores).
- Output is stored as bf16 (halves store traffic; host upcasts).
End-to-end rel err ~4.7e-3.
"""

import numpy as np

try:
    import concourse.bass as bass  # noqa: F401
except ImportError:
    import sys

    sys.path.insert(0, "/opt/trn_rl_repo")

import ml_dtypes

import concourse.mybir as mybir
import concourse.tile as tile
from concourse import bacc
from concourse.bass_utils import run_bass_kernel_spmd

F32 = mybir.dt.float32
BF16 = mybir.dt.bfloat16
SILU = mybir.ActivationFunctionType.Silu
P = 128
NPBF = ml_dtypes.bfloat16

NCORES = 8
H_FULL = 2048  # hidden size
D_FULL = 2048  # expert intermediate size
T_TOTAL = 32768
T_CORE = T_TOTAL // NCORES  # 4096 tokens per expert/core


def emit_moe(nc, out_ap, xt_ap, w1_ap, w1pair_ap, w2_ap, T, H, D, TC):
    K1 = H // P  # contraction tiles for mm1
    MP = D // P  # gate/up column-block pairs
    K2 = D // P  # contraction tiles for mm2
    NCH = T // TC  # token chunks
    MMW = 512  # moving width = one PSUM bank of f32
    NHF = TC // MMW  # 512-wide column groups per chunk (2)
    NHQ = H // MMW  # mm2 output column groups (4)
    NTB = TC // P  # token blocks per chunk for mm2 (8)

    KG = 4  # k-tiles per xT sub-tile (split so mm1 starts after 1/4 of x)
    NKG = K1 // KG

    with tile.TileContext(nc) as tc:
        with (
            tc.tile_pool(name="w2sb", bufs=1) as w2p,
            tc.tile_pool(name="w1pair", bufs=1) as w1pp,
            tc.tile_pool(name="const", bufs=1) as constp,
            tc.tile_pool(name="xT", bufs=NKG) as xTp,
            tc.tile_pool(name="actT", bufs=1) as actTp,
            tc.tile_pool(name="w1", bufs=6) as w1p,
            tc.tile_pool(name="sil", bufs=2 * NHF) as silp,
            tc.tile_pool(name="ost", bufs=2) as ostp,
            tc.tile_pool(name="ps", bufs=8, space="PSUM") as psp,
        ):
            # ---- load helpers with explicit prefetch scheduling ----
            xts = {}

            def load_xt_kg(c, kg):
                t = xTp.tile([P, KG * TC], BF16, tag="xT", name=f"xT_{c}_{kg}")
                nc.sync.dma_start(
                    out=t[:].rearrange("p (k t) -> p k t", k=KG),
                    in_=xt_ap[c, :, kg * KG : (kg + 1) * KG, :],
                )
                xts[(c, kg)] = t

            def load_xt_kg_split(c, kg):
                # chunk-0 startup: two 2-k-tile halves as separate sync-ring
                # DMAs, so the PE's first matmuls gate on 0.5 MB, not 1 MB.
                # (Per-core HBM read bw is shared across rings, so spreading
                # these over gpsimd buys nothing — measured.)
                t = xTp.tile([P, KG * TC], BF16, tag="xT", name=f"xT_{c}_{kg}")
                hk = KG // 2
                for h in range(2):
                    nc.sync.dma_start(
                        out=t[:, h * hk * TC : (h + 1) * hk * TC].rearrange(
                            "p (k t) -> p k t", k=hk
                        ),
                        in_=xt_ap[c, :, kg * KG + h * hk : kg * KG + (h + 1) * hk, :],
                    )
                xts[(c, kg)] = t

            def load_xt(c):
                if c >= NCH:
                    return
                for kg in range(NKG):
                    load_xt_kg(c, kg)

            w1s = {}

            def load_w1_one(c, mp, gu):
                t = w1p.tile([P, K1 * P], BF16, tag="w1", name=f"w1_{c}_{mp}_{gu}")
                nc.sync.dma_start(out=t[:], in_=w1_ap[mp * 2 + gu])
                w1s[(c, mp, gu)] = t

            def load_w1(c, mp):
                if c >= NCH or mp >= MP:
                    return
                load_w1_one(c, mp, 0)
                load_w1_one(c, mp, 1)

            # Startup order matters. Chunk 0's first TWO mp iterations run
            # k-outer with gate+up for both mps accumulating across all 8
            # PSUM banks at once (see below), which drops the PE's x
            # appetite during the supply-critical window to 0.145 MB/us —
            # well under the ~0.42 MB/us ring — so the stream runs
            # stall-free from its first matmul. The ring order matches the
            # k-outer consumption: the four k0-7 stationary halves first,
            # then x in k order (single-k first pieces to open the gate
            # early), then the k8-15 halves, then the rest.
            # The startup stationaries ride a host-packed contiguous tensor
            # (w1pair: both k-halves of mp0/mp1 gate+up) because slicing
            # the regular w1 tiles into k-halves makes 2 KB-per-partition
            # strided DMAs that run at HALF ring rate (measured: it blew
            # the data gate from 12.3 us to 14.8 us and reset the clock).
            # Four 4-k pieces ([128,2048] = 4 KB/partition, full ring rate)
            # interleaved with the x halves in k order: the data gate for
            # the first matmul is w1pair piece 0 + x(k0,k1) = 1 MB.
            QWC = 4 * 4 * P  # 2048 columns per packed 4-k piece
            w1pair = w1pp.tile([P, 4 * QWC], BF16, tag="w1pair", name="w1pair")
            nc.sync.dma_start(out=w1pair[:, :QWC], in_=w1pair_ap[0])
            load_xt_kg_split(0, 0)
            nc.sync.dma_start(out=w1pair[:, QWC : 2 * QWC], in_=w1pair_ap[1])
            load_xt_kg_split(0, 1)
            nc.sync.dma_start(out=w1pair[:, 2 * QWC : 3 * QWC], in_=w1pair_ap[2])
            nc.sync.dma_start(out=w1pair[:, 3 * QWC :], in_=w1pair_ap[3])
            load_xt_kg_split(0, 2)
            load_xt_kg_split(0, 3)

            def pair_st(mp, gu, k):
                # stationary [128,128] for the paired pass
                base = (k // 4) * QWC + ((mp * 2 + gu) * 4 + k % 4) * P
                return w1pair[:, base : base + P]
            w2sb = w2p.tile([P, K2 * H], BF16, tag="w2", name="w2sb")

            def load_w2_quarter(q):
                # W2 is 8MB; issued whole (or early) it starves the chunk-0
                # x/w1 loads on the HBM-bandwidth-shared rings, so it is
                # threaded between mm1(0) weight loads on the sync ring.
                kq = K2 // 4
                nc.sync.dma_start(
                    out=w2sb[:, q * kq * H : (q + 1) * kq * H],
                    in_=w2_ap[:, q * kq : (q + 1) * kq, :].rearrange(
                        "p k h -> p (k h)"
                    ),
                )

            # Warm-up matmuls on a zero tile: they depend on nothing, so
            # they run during the initial DMAs and pull the PE out of its
            # low p-state right as the real stream begins. Sizing is
            # load-bearing, measured both ways: the DVFS ramp needs ~3.6 us
            # of busy PE (8 warm-ups at 427 ns mid-clock) or the first real
            # matmuls run at mid clock; and any PE idle gap over ~2 us
            # RESETS the clock to mid, so the stream must not start before
            # the x supply can sustain it. Delaying the warm-ups (e.g.
            # gating them on the w1g DMA) does NOT shrink the measured
            # window: the profiler's exec window opens at the framework's
            # own constant-pool memsets (~6.1 us), so the only objective
            # is the absolute end time -> start the ramp as early as
            # possible.
            zc = constp.tile([P, MMW], BF16, tag="zc", name="zc")
            nc.gpsimd.memset(zc[:], 0.0)
            psw = psp.tile([P, MMW], F32, tag="ps", name="psw")
            for i in range(10):
                nc.tensor.matmul(
                    psw[:], zc[:, :P], zc[:], start=True, stop=True
                )

            for c in range(NCH):
                t0 = c * TC
                xT = {kg: xts.pop((c, kg)) for kg in range(NKG)}

                # ---- mm1 + SwiGLU -> actT [D on partitions, T free] ----
                actT = actTp.tile([P, K2 * TC], BF16, tag="actT", name=f"actT_{c}")
                mp_start = 0
                if c == 0:
                    # Paired k-outer startup: mp 0 and 1, gate and up, both
                    # hf groups accumulate together — 8 live PSUM banks, 8
                    # matmuls (1.73 us) per 0.25 MB x k-tile, so the PE
                    # never outruns the DMA ring during startup.
                    mp_start = 2
                    pps = {
                        (mp, gu, hf): psp.tile(
                            [P, MMW], F32, tag="ps", name=f"pp{mp}{gu}{hf}"
                        )
                        for mp in (0, 1)
                        for gu in (0, 1)
                        for hf in range(NHF)
                    }
                    for k in range(K1):
                        if k == 8:
                            # lookahead-2 weight prefetch for mp 2 and 3,
                            # queued behind the startup-critical loads
                            load_w1(0, 2)
                            load_w1(0, 3)
                        xk = xT[k // KG]
                        kk = k % KG
                        for mp in (0, 1):
                            for gu in (0, 1):
                                st = pair_st(mp, gu, k)
                                for hf in range(NHF):
                                    nc.tensor.matmul(
                                        pps[(mp, gu, hf)][:],
                                        st,
                                        xk[
                                            :,
                                            kk * TC
                                            + hf * MMW : kk * TC
                                            + (hf + 1) * MMW,
                                        ],
                                        start=(k == 0),
                                        stop=(k == K1 - 1),
                                    )
                    for mp in (0, 1):
                        sil = [
                            silp.tile([P, MMW], BF16, tag="sil", name=f"psil{mp}{i}")
                            for i in range(NHF)
                        ]
                        for hf in range(NHF):
                            nc.scalar.activation(sil[hf][:], pps[(mp, 0, hf)][:], SILU)
                        for hf in range(NHF):
                            nc.vector.tensor_mul(
                                actT[:, mp * TC + hf * MMW : mp * TC + (hf + 1) * MMW],
                                sil[hf][:],
                                pps[(mp, 1, hf)][:],
                            )
                for mp in range(mp_start, MP):
                    load_w1(c, mp + 2)
                    if c == 0 and mp in (6, 8, 10, 12):
                        load_w2_quarter((mp - 6) // 2)
                    w1g = w1s.pop((c, mp, 0))
                    w1u = w1s.pop((c, mp, 1))
                    psg = [psp.tile([P, MMW], F32, tag="ps", name=f"psg{i}") for i in range(NHF)]
                    psu = [psp.tile([P, MMW], F32, tag="ps", name=f"psu{i}") for i in range(NHF)]
                    for k in range(K1):
                        st = w1g[:, k * P : (k + 1) * P]
                        xk = xT[k // KG]
                        kk = k % KG
                        for hf in range(NHF):
                            nc.tensor.matmul(
                                psg[hf][:],
                                st,
                                xk[:, kk * TC + hf * MMW : kk * TC + (hf + 1) * MMW],
                                start=(k == 0),
                                stop=(k == K1 - 1),
                            )
                    sil = [silp.tile([P, MMW], BF16, tag="sil", name=f"sil{i}") for i in range(NHF)]
                    for hf in range(NHF):
                        nc.scalar.activation(sil[hf][:], psg[hf][:], SILU)
                    for k in range(K1):
                        st = w1u[:, k * P : (k + 1) * P]
                        xk = xT[k // KG]
                        kk = k % KG
                        for hf in range(NHF):
                            nc.tensor.matmul(
                                psu[hf][:],
                                st,
                                xk[:, kk * TC + hf * MMW : kk * TC + (hf + 1) * MMW],
                                start=(k == 0),
                                stop=(k == K1 - 1),
                            )
                    for hf in range(NHF):
                        nc.vector.tensor_mul(
                            actT[:, mp * TC + hf * MMW : mp * TC + (hf + 1) * MMW],
                            sil[hf][:],
                            psu[hf][:],
                        )

                # prefetch next chunk's x and first weight pairs while the
                # PE runs mm2 on this chunk
                load_xt(c + 1)
                load_w1(c + 1, 0)
                load_w1(c + 1, 1)

                # ---- mm2: actT stationary, W2 moving -> natural [t, H] ----
                # hq-outer: each [128,512] output group's accumulation
                # retires 16 matmuls before the next, so PSUM evacuation
                # (and on the final block, the store) trails the stream by
                # only ~0.7 us instead of a whole tb.
                last_blk = c == NCH - 1
                for tb in range(NTB):
                    last_tb = last_blk and tb == NTB - 1
                    ost = ostp.tile([P, H], BF16, tag="ost", name=f"ost{tb}")
                    for hq in range(NHQ):
                        ps2 = psp.tile([P, MMW], F32, tag="ps", name=f"ps2_{hq}")
                        for k2 in range(K2):
                            st = actT[:, k2 * TC + tb * P : k2 * TC + (tb + 1) * P]
                            nc.tensor.matmul(
                                ps2[:],
                                st,
                                w2sb[:, k2 * H + hq * MMW : k2 * H + (hq + 1) * MMW],
                                start=(k2 == 0),
                                stop=(k2 == K2 - 1),
                            )
                        dst = ost[:, hq * MMW : (hq + 1) * MMW]
                        if last_tb:
                            # final block: copies split across ACT/DVE and
                            # stores split across the scalar/sync rings so
                            # the kernel tail drains behind the last matmul
                            if hq == NHQ - 1:
                                # very last group: half-copies run on BOTH
                                # engines in parallel and the halves store
                                # on both rings — shortest possible drain
                                hm = MMW // 2
                                nc.vector.tensor_copy(dst[:, :hm], ps2[:, :hm])
                                nc.scalar.copy(dst[:, hm:], ps2[:, hm:])
                                rows = slice(t0 + tb * P, t0 + (tb + 1) * P)
                                nc.scalar.dma_start(
                                    out=out_ap[rows, hq * MMW : hq * MMW + hm],
                                    in_=dst[:, :hm],
                                )
                                nc.sync.dma_start(
                                    out=out_ap[rows, hq * MMW + hm : (hq + 1) * MMW],
                                    in_=dst[:, hm:],
                                )
                                continue
                            if hq % 2 == 0:
                                nc.vector.tensor_copy(dst, ps2[:])
                            else:
                                nc.scalar.copy(dst, ps2[:])
                            eng = nc.scalar if hq % 2 == 0 else nc.sync
                            eng.dma_start(
                                out=out_ap[
                                    t0 + tb * P : t0 + (tb + 1) * P,
                                    hq * MMW : (hq + 1) * MMW,
                                ],
                                in_=dst,
                            )
                        else:
                            # copies go on the DVE (idle during mm2) so the
                            # ACT queue holds only sils and the next chunk's
                            # mm1 never waits on a sil stuck behind these
                            nc.vector.tensor_copy(dst, ps2[:])
                    if not last_tb:
                        nc.scalar.dma_start(
                            out=out_ap[t0 + tb * P : t0 + (tb + 1) * P, :], in_=ost[:]
                        )


def build(T=T_CORE, H=H_FULL, D=D_FULL, TC=1024):
    nc = bacc.Bacc("TRN2", target_bir_lowering=False, debug=False)
    xt = nc.dram_tensor(
        "xt", [T // TC, P, H // P, TC], BF16, kind="ExternalInput"
    ).ap()
    w1 = nc.dram_tensor(
        "w1", [2 * D // P, P, H // P * P], BF16, kind="ExternalInput"
    ).ap()
    w1pair = nc.dram_tensor(
        "w1pair", [4, P, 4 * 4 * P], BF16, kind="ExternalInput"
    ).ap()
    w2 = nc.dram_tensor("w2", [P, D // P, H], BF16, kind="ExternalInput").ap()
    out = nc.dram_tensor("out", [T, H], BF16, kind="ExternalOutput").ap()
    emit_moe(nc, out, xt, w1, w1pair, w2, T, H, D, TC)
    nc.compile()
    return nc


_NC_CACHE = {}


def _get_nc():
    if "nc" not in _NC_CACHE:
        _NC_CACHE["nc"] = build()
    return _NC_CACHE["nc"]


def _prep_core(args):
    """Host-side pack of one expert's operands into device layouts."""
    x_e, w1_e, w2_e = args
    T, H, D = T_CORE, H_FULL, D_FULL
    # xt[c, p, k, t] = x[c*TC + t, k*128+p], chunk-major so each chunk's
    # per-partition DMA segments are KG*TC contiguous
    TC = 1024
    xbf = x_e.astype(NPBF)
    xt = np.ascontiguousarray(
        xbf.reshape(T // TC, TC, H // P, P).transpose(0, 3, 2, 1)
    )
    # w1r[mp, gu, p, k, c] = W1[k*128+p, gu*D + mp*128 + c], flattened to
    # [32, 128, 2048]
    w1bf = w1_e.astype(NPBF)
    w1r = np.ascontiguousarray(
        w1bf.reshape(H // P, P, 2, D // P, P).transpose(3, 2, 1, 0, 4)
    ).reshape(2 * D // P, P, H // P * P)
    # w1pair[q, p, (j*4 + kk)*128 + c] = w1r tile j's k-tile 4q+kk
    # (j = mp*2+gu, mp<2): the startup stationaries packed contiguously
    # in 4-k pieces so their DMAs move 4 KB per partition (full ring
    # rate) and the first piece + first x half opens the stream gate.
    w1pair = np.ascontiguousarray(
        w1r[:4].reshape(4, P, 4, 4, P).transpose(2, 1, 0, 3, 4)
    ).reshape(4, P, 4 * 4 * P)
    # w2r[p, k2, h] = W2[k2*128+p, h]
    w2bf = w2_e.astype(NPBF)
    w2r = np.ascontiguousarray(w2bf.reshape(D // P, P, H).transpose(1, 0, 2))
    return {"xt": xt, "w1": w1r, "w1pair": w1pair, "w2": w2r}


def run_sharded(hidden_states, gate_up_proj, down_proj, trace=False, **kwargs):
    """Run on 8 cores; returns (full_output, BassKernelResults)."""
    hidden_states = np.asarray(hidden_states, dtype=np.float32)
    gate_up_proj = np.asarray(gate_up_proj, dtype=np.float32)
    down_proj = np.asarray(down_proj, dtype=np.float32)

    nc = _get_nc()
    in_maps = [
        _prep_core(
            (
                hidden_states[e * T_CORE : (e + 1) * T_CORE],
                gate_up_proj[e],
                down_proj[e],
            )
        )
        for e in range(NCORES)
    ]
    res = run_bass_kernel_spmd(
        nc, in_maps, core_ids=list(range(NCORES)), trace=trace, **kwargs
    )
    out = np.concatenate(
        [res.results[e]["out"].astype(np.float32) for e in range(NCORES)], axis=0
    )
    return out, res


def kernel(hidden_states, gate_up_proj, down_proj):
    import os

    # The NTFF trace path needs antenv.axon_hooks, absent in this image;
    # make sure a stray BASS_TRACE env can't route us into it.
    os.environ["BASS_NEVER_TRACE"] = "1"
    try:
        # NOTE: do NOT run the NEFF twice for DVFS warm-up, however
        # tempting (cold device = 1.61 ms vs warm 1.345 ms): a second
        # execution produces a second per-core NTFF and
        # gauge.profiler.convert_ntffs_to_json ASSERTS on multiple
        # ntffs mapping to one json path — it would crash the profiling
        # harness entirely.
        out, _ = run_sharded(hidden_states, gate_up_proj, down_proj)
    finally:
        del os.environ["BASS_NEVER_TRACE"]
    return out


# revision 40
# speedup vs baseline: 1.1997x; 1.1997x over previous
"""Llama4TextExperts MoE grouped-GEMM kernel for 8 Trainium2 NeuronCores.

Expert-parallel: core e owns expert e and the pre-sorted token block
hidden_states[e*4096:(e+1)*4096]. No collectives needed.

The host pre-packs all operands into bf16 device layouts so the PE does
nothing but the 6144 GEMM matmuls (no on-chip transposes):
  xt [4, 128, 16, 1024]  xt[c,p,k,t] = x[c*1024+t, k*128+p]  (xT, chunked)
  w1 [32, 128, 2048]     w1[mp*2+gu] = W1 column block, k-tiled, contiguous
  w2 [128, 16, 2048]     w2[p,k2,h]  = W2[k2*128+p, h]

Per-core pipeline over 4 token chunks of TC=1024:
  mm1: for each of 16 gate/up column-block pairs, accumulate
       psg/psu [128,512] over k=16 (bf16 matmuls, f32 PSUM),
       ACT silu -> bf16, DVE mul -> actT bf16 [D-part, T free]
  mm2: actT block as stationary, W2 rows as moving -> natural [token, H]
       PSUM output; DVE copy -> bf16 SBUF, store DMA on the scalar queue.

Scheduling notes (all trace-verified):
- The PE stream runs at 216 ns per 512-wide matmul (512 rows at the
  2.37 GHz DVFS-capped clock; LDWEIGHTS hidden). The optimization
  budget is only the ~20 us outside the stream.
- Startup: the first load packet lands ~8.7 us (fixed preamble), and
  per-core HBM read bandwidth is ~0.42 MB/us SHARED across all DMA
  rings (splitting loads over sync+gpsimd was measured to give zero
  aggregate gain). A plain mp-at-a-time mm1 consumes x at 0.59 MB/us
  and stalls on the ring; chunk 0 therefore runs its first TWO mp
  iterations k-outer with gate+up for both mps accumulating across
  all 8 PSUM banks at once, dropping consumption to 0.145 MB/us —
  the stream runs stall-free, which also removes the DVFS-reset
  failure mode on slow-supply runs. W2's 8 MB is threaded in
  quarters into the mp=6..12 weight loads where the ring has slack.
- 10 warm-up matmuls on a zero tile ride the initial DMA wait and pull
  the PE out of its low p-state right as the real stream begins (the
  DVFS ramp needs ~3.6 us of busy PE; an idle gap > ~2 us resets it —
  the stream start is data-walled at ~13.5 us, so the extra warm-ups
  are reset-insurance, not delay).
- The startup stationaries ride a host-packed contiguous tensor:
  k-half slices of the regular w1 tiles make 2 KB-per-partition
  strided DMAs that run at HALF ring rate (measured +2.5 us gate).
- mm2 runs hq-outer so each [128,512] output group retires 3.5 us
  before the next: the final block's copies (split ACT/DVE) and
  stores (split scalar/sync rings) drain right behind the last matmul
  (tail measured 5.2 us vs 7.5 us for tb-batched stores).
- Output is stored as bf16 (halves store traffic; host upcasts).
End-to-end rel err ~4.7e-3.
"""

import numpy as np

try:
    import concourse.bass as bass  # noqa: F401
except ImportError:
    import sys

    sys.path.insert(0, "/opt/trn_rl_repo")

import ml_dtypes

import concourse.mybir as mybir
import concourse.tile as tile
from concourse import bacc
from concourse.bass_utils import run_bass_kernel_spmd

F32 = mybir.dt.float32
BF16 = mybir.dt.bfloat16
SILU = mybir.ActivationFunctionType.Silu
P = 128
NPBF = ml_dtypes.bfloat16

NCORES = 8
H_FULL = 2048  # hidden size
D_FULL = 2048  # expert intermediate size
T_TOTAL = 32768
T_CORE = T_TOTAL // NCORES  # 4096 tokens per expert/core


def emit_moe(nc, out_ap, xt_ap, w1_ap, w1pair_ap, w2_ap, T, H, D, TC):
    K1 = H // P  # contraction tiles for mm1
    MP = D // P  # gate/up column-block pairs
    K2 = D // P  # contraction tiles for mm2
    NCH = T // TC  # token chunks
    MMW = 512  # moving width = one PSUM bank of f32
    NHF = TC // MMW  # 512-wide column groups per chunk (2)
    NHQ = H // MMW  # mm2 output column groups (4)
    NTB = TC // P  # token blocks per chunk for mm2 (8)

    KG = 4  # k-tiles per xT sub-tile (split so mm1 starts after 1/4 of x)
    NKG = K1 // KG

    with tile.TileContext(nc) as tc:
        with (
            tc.tile_pool(name="w2sb", bufs=1) as w2p,
            tc.tile_pool(name="w1pair", bufs=1) as w1pp,
            tc.tile_pool(name="const", bufs=1) as constp,
            tc.tile_pool(name="xT", bufs=NKG) as xTp,
            tc.tile_pool(name="actT", bufs=1) as actTp,
            tc.tile_pool(name="w1", bufs=6) as w1p,
            tc.tile_pool(name="sil", bufs=2 * NHF) as silp,
            tc.tile_pool(name="ost", bufs=2) as ostp,
            tc.tile_pool(name="ps", bufs=8, space="PSUM") as psp,
        ):
            # ---- load helpers with explicit prefetch scheduling ----
            xts = {}

            def load_xt_kg(c, kg):
                t = xTp.tile([P, KG * TC], BF16, tag="xT", name=f"xT_{c}_{kg}")
                nc.sync.dma_start(
                    out=t[:].rearrange("p (k t) -> p k t", k=KG),
                    in_=xt_ap[c, :, kg * KG : (kg + 1) * KG, :],
                )
                xts[(c, kg)] = t

            def load_xt_kg_split(c, kg):
                # chunk-0 startup: two 2-k-tile halves as separate sync-ring
                # DMAs, so the PE's first matmuls gate on 0.5 MB, not 1 MB.
                # (Per-core HBM read bw is shared across rings, so spreading
                # these over gpsimd buys nothing — measured.)
                t = xTp.tile([P, KG * TC], BF16, tag="xT", name=f"xT_{c}_{kg}")
                hk = KG // 2
                for h in range(2):
                    nc.sync.dma_start(
                        out=t[:, h * hk * TC : (h + 1) * hk * TC].rearrange(
                            "p (k t) -> p k t", k=hk
                        ),
                        in_=xt_ap[c, :, kg * KG + h * hk : kg * KG + (h + 1) * hk, :],
                    )
                xts[(c, kg)] = t

            def load_xt(c):
                if c >= NCH:
                    return
                for kg in range(NKG):
                    load_xt_kg(c, kg)

            w1s = {}

            def load_w1_one(c, mp, gu):
                t = w1p.tile([P, K1 * P], BF16, tag="w1", name=f"w1_{c}_{mp}_{gu}")
                nc.sync.dma_start(out=t[:], in_=w1_ap[mp * 2 + gu])
                w1s[(c, mp, gu)] = t

            def load_w1(c, mp):
                if c >= NCH or mp >= MP:
                    return
                load_w1_one(c, mp, 0)
                load_w1_one(c, mp, 1)

            # Startup order matters. Chunk 0's first TWO mp iterations run
            # k-outer with gate+up for both mps accumulating across all 8
            # PSUM banks at once (see below), which drops the PE's x
            # appetite during the supply-critical window to 0.145 MB/us —
            # well under the ~0.42 MB/us ring — so the stream runs
            # stall-free from its first matmul. The ring order matches the
            # k-outer consumption: the four k0-7 stationary halves first,
            # then x in k order (single-k first pieces to open the gate
            # early), then the k8-15 halves, then the rest.
            # The startup stationaries ride a host-packed contiguous tensor
            # (w1pair: both k-halves of mp0/mp1 gate+up) because slicing
            # the regular w1 tiles into k-halves makes 2 KB-per-partition
            # strided DMAs that run at HALF ring rate (measured: it blew
            # the data gate from 12.3 us to 14.8 us and reset the clock).
            # Four 4-k pieces ([128,2048] = 4 KB/partition, full ring rate)
            # interleaved with the x halves in k order: the data gate for
            # the first matmul is w1pair piece 0 + x(k0,k1) = 1 MB.
            QWC = 4 * 4 * P  # 2048 columns per packed 4-k piece
            w1pair = w1pp.tile([P, 4 * QWC], BF16, tag="w1pair", name="w1pair")
            nc.sync.dma_start(out=w1pair[:, :QWC], in_=w1pair_ap[0])
            load_xt_kg_split(0, 0)
            nc.sync.dma_start(out=w1pair[:, QWC : 2 * QWC], in_=w1pair_ap[1])
            load_xt_kg_split(0, 1)
            nc.sync.dma_start(out=w1pair[:, 2 * QWC : 3 * QWC], in_=w1pair_ap[2])
            nc.sync.dma_start(out=w1pair[:, 3 * QWC :], in_=w1pair_ap[3])
            load_xt_kg_split(0, 2)
            load_xt_kg_split(0, 3)

            def pair_st(mp, gu, k):
                # stationary [128,128] for the paired pass
                base = (k // 4) * QWC + ((mp * 2 + gu) * 4 + k % 4) * P
                return w1pair[:, base : base + P]
            w2sb = w2p.tile([P, K2 * H], BF16, tag="w2", name="w2sb")

            def load_w2_quarter(q):
                # W2 is 8MB; issued whole (or early) it starves the chunk-0
                # x/w1 loads on the HBM-bandwidth-shared rings, so it is
                # threaded between mm1(0) weight loads on the sync ring.
                kq = K2 // 4
                nc.sync.dma_start(
                    out=w2sb[:, q * kq * H : (q + 1) * kq * H],
                    in_=w2_ap[:, q * kq : (q + 1) * kq, :].rearrange(
                        "p k h -> p (k h)"
                    ),
                )

            # Warm-up matmuls on a zero tile: they depend on nothing, so
            # they run during the initial DMAs and pull the PE out of its
            # low p-state right as the real stream begins. Sizing is
            # load-bearing, measured both ways: the DVFS ramp needs ~3.6 us
            # of busy PE (8 warm-ups at 427 ns mid-clock) or the first real
            # matmuls run at mid clock; and any PE idle gap over ~2 us
            # RESETS the clock to mid, so the stream must not start before
            # the x supply can sustain it. Delaying the warm-ups (e.g.
            # gating them on the w1g DMA) does NOT shrink the measured
            # window: the profiler's exec window opens at the framework's
            # own constant-pool memsets (~6.1 us), so the only objective
            # is the absolute end time -> start the ramp as early as
            # possible.
            zc = constp.tile([P, MMW], BF16, tag="zc", name="zc")
            nc.gpsimd.memset(zc[:], 0.0)
            # 13 warm-ups: 8 cover the DVFS ramp at 427 ns, the rest run
            # at full clock and end (~12.7 us) just before the earliest
            # observed data gate (13.4 us) — zero cost on fast-supply
            # runs, and on slow-supply runs they keep the pre-stream
            # idle under the ~2 us clock-reset threshold (a measured
            # 2.9 us idle cost ~3 us of mid-clock stream).
            psw = psp.tile([P, MMW], F32, tag="ps", name="psw")
            for i in range(13):
                nc.tensor.matmul(
                    psw[:], zc[:, :P], zc[:], start=True, stop=True
                )

            for c in range(NCH):
                t0 = c * TC
                xT = {kg: xts.pop((c, kg)) for kg in range(NKG)}

                # ---- mm1 + SwiGLU -> actT [D on partitions, T free] ----
                actT = actTp.tile([P, K2 * TC], BF16, tag="actT", name=f"actT_{c}")
                mp_start = 0
                if c == 0:
                    # Paired k-outer startup: mp 0 and 1, gate and up, both
                    # hf groups accumulate together — 8 live PSUM banks, 8
                    # matmuls (1.73 us) per 0.25 MB x k-tile, so the PE
                    # never outruns the DMA ring during startup.
                    mp_start = 2
                    pps = {
                        (mp, gu, hf): psp.tile(
                            [P, MMW], F32, tag="ps", name=f"pp{mp}{gu}{hf}"
                        )
                        for mp in (0, 1)
                        for gu in (0, 1)
                        for hf in range(NHF)
                    }
                    for k in range(K1):
                        if k == 8:
                            # lookahead-2 weight prefetch for mp 2 and 3,
                            # queued behind the startup-critical loads
                            load_w1(0, 2)
                            load_w1(0, 3)
                        xk = xT[k // KG]
                        kk = k % KG
                        for mp in (0, 1):
                            for gu in (0, 1):
                                st = pair_st(mp, gu, k)
                                for hf in range(NHF):
                                    nc.tensor.matmul(
                                        pps[(mp, gu, hf)][:],
                                        st,
                                        xk[
                                            :,
                                            kk * TC
                                            + hf * MMW : kk * TC
                                            + (hf + 1) * MMW,
                                        ],
                                        start=(k == 0),
                                        stop=(k == K1 - 1),
                                    )
                    for mp in (0, 1):
                        sil = [
                            silp.tile([P, MMW], BF16, tag="sil", name=f"psil{mp}{i}")
                            for i in range(NHF)
                        ]
                        for hf in range(NHF):
                            nc.scalar.activation(sil[hf][:], pps[(mp, 0, hf)][:], SILU)
                        for hf in range(NHF):
                            nc.vector.tensor_mul(
                                actT[:, mp * TC + hf * MMW : mp * TC + (hf + 1) * MMW],
                                sil[hf][:],
                                pps[(mp, 1, hf)][:],
                            )
                for mp in range(mp_start, MP):
                    load_w1(c, mp + 2)
                    if c == 0 and mp in (6, 8, 10, 12):
                        load_w2_quarter((mp - 6) // 2)
                    w1g = w1s.pop((c, mp, 0))
                    w1u = w1s.pop((c, mp, 1))
                    psg = [psp.tile([P, MMW], F32, tag="ps", name=f"psg{i}") for i in range(NHF)]
                    psu = [psp.tile([P, MMW], F32, tag="ps", name=f"psu{i}") for i in range(NHF)]
                    for k in range(K1):
                        st = w1g[:, k * P : (k + 1) * P]
                        xk = xT[k // KG]
                        kk = k % KG
                        for hf in range(NHF):
                            nc.tensor.matmul(
                                psg[hf][:],
                                st,
                                xk[:, kk * TC + hf * MMW : kk * TC + (hf + 1) * MMW],
                                start=(k == 0),
                                stop=(k == K1 - 1),
                            )
                    sil = [silp.tile([P, MMW], BF16, tag="sil", name=f"sil{i}") for i in range(NHF)]
                    for hf in range(NHF):
                        nc.scalar.activation(sil[hf][:], psg[hf][:], SILU)
                    for k in range(K1):
                        st = w1u[:, k * P : (k + 1) * P]
                        xk = xT[k // KG]
                        kk = k % KG
                        for hf in range(NHF):
                            nc.tensor.matmul(
                                psu[hf][:],
                                st,
                                xk[:, kk * TC + hf * MMW : kk * TC + (hf + 1) * MMW],
                                start=(k == 0),
                                stop=(k == K1 - 1),
                            )
                    for hf in range(NHF):
                        nc.vector.tensor_mul(
                            actT[:, mp * TC + hf * MMW : mp * TC + (hf + 1) * MMW],
                            sil[hf][:],
                            psu[hf][:],
                        )

                # prefetch next chunk's x and first weight pairs while the
                # PE runs mm2 on this chunk
                load_xt(c + 1)
                load_w1(c + 1, 0)
                load_w1(c + 1, 1)

                # ---- mm2: actT stationary, W2 moving -> natural [t, H] ----
                # hq-outer: each [128,512] output group's accumulation
                # retires 16 matmuls before the next, so PSUM evacuation
                # (and on the final block, the store) trails the stream by
                # only ~0.7 us instead of a whole tb.
                last_blk = c == NCH - 1
                for tb in range(NTB):
                    last_tb = last_blk and tb == NTB - 1
                    ost = ostp.tile([P, H], BF16, tag="ost", name=f"ost{tb}")
                    for hq in range(NHQ):
                        ps2 = psp.tile([P, MMW], F32, tag="ps", name=f"ps2_{hq}")
                        for k2 in range(K2):
                            st = actT[:, k2 * TC + tb * P : k2 * TC + (tb + 1) * P]
                            nc.tensor.matmul(
                                ps2[:],
                                st,
                                w2sb[:, k2 * H + hq * MMW : k2 * H + (hq + 1) * MMW],
                                start=(k2 == 0),
                                stop=(k2 == K2 - 1),
                            )
                        dst = ost[:, hq * MMW : (hq + 1) * MMW]
                        if last_tb:
                            # final block: copies split across ACT/DVE and
                            # stores split across the scalar/sync rings so
                            # the kernel tail drains behind the last matmul
                            if hq == NHQ - 1:
                                # very last group: half-copies run on BOTH
                                # engines in parallel and the halves store
                                # on both rings — shortest possible drain
                                hm = MMW // 2
                                nc.vector.tensor_copy(dst[:, :hm], ps2[:, :hm])
                                nc.scalar.copy(dst[:, hm:], ps2[:, hm:])
                                rows = slice(t0 + tb * P, t0 + (tb + 1) * P)
                                nc.scalar.dma_start(
                                    out=out_ap[rows, hq * MMW : hq * MMW + hm],
                                    in_=dst[:, :hm],
                                )
                                nc.sync.dma_start(
                                    out=out_ap[rows, hq * MMW + hm : (hq + 1) * MMW],
                                    in_=dst[:, hm:],
                                )
                                continue
                            if hq % 2 == 0:
                                nc.vector.tensor_copy(dst, ps2[:])
                            else:
                                nc.scalar.copy(dst, ps2[:])
                            eng = nc.scalar if hq % 2 == 0 else nc.sync
                            eng.dma_start(
                                out=out_ap[
                                    t0 + tb * P : t0 + (tb + 1) * P,
                                    hq * MMW : (hq + 1) * MMW,
                                ],
                                in_=dst,
                            )
                        else:
                            # copies go on the DVE (idle during mm2) so the
                            # ACT queue holds only sils and the next chunk's
                            # mm1 never waits on a sil stuck behind these
                            nc.vector.tensor_copy(dst, ps2[:])
                    if not last_tb:
                        nc.scalar.dma_start(
                            out=out_ap[t0 + tb * P : t0 + (tb + 1) * P, :], in_=ost[:]
                        )


def build(T=T_CORE, H=H_FULL, D=D_FULL, TC=1024):
    nc = bacc.Bacc("TRN2", target_bir_lowering=False, debug=False)
    xt = nc.dram_tensor(
        "xt", [T // TC, P, H // P, TC], BF16, kind="ExternalInput"
    ).ap()
    w1 = nc.dram_tensor(
        "w1", [2 * D // P, P, H // P * P], BF16, kind="ExternalInput"
    ).ap()
    w1pair = nc.dram_tensor(
        "w1pair", [4, P, 4 * 4 * P], BF16, kind="ExternalInput"
    ).ap()
    w2 = nc.dram_tensor("w2", [P, D // P, H], BF16, kind="ExternalInput").ap()
    out = nc.dram_tensor("out", [T, H], BF16, kind="ExternalOutput").ap()
    emit_moe(nc, out, xt, w1, w1pair, w2, T, H, D, TC)
    nc.compile()
    return nc


_NC_CACHE = {}


def _get_nc():
    if "nc" not in _NC_CACHE:
        _NC_CACHE["nc"] = build()
    return _NC_CACHE["nc"]


def _prep_core(args):
    """Host-side pack of one expert's operands into device layouts."""
    x_e, w1_e, w2_e = args
    T, H, D = T_CORE, H_FULL, D_FULL
    # xt[c, p, k, t] = x[c*TC + t, k*128+p], chunk-major so each chunk's
    # per-partition DMA segments are KG*TC contiguous
    TC = 1024
    xbf = x_e.astype(NPBF)
    xt = np.ascontiguousarray(
        xbf.reshape(T // TC, TC, H // P, P).transpose(0, 3, 2, 1)
    )
    # w1r[mp, gu, p, k, c] = W1[k*128+p, gu*D + mp*128 + c], flattened to
    # [32, 128, 2048]
    w1bf = w1_e.astype(NPBF)
    w1r = np.ascontiguousarray(
        w1bf.reshape(H // P, P, 2, D // P, P).transpose(3, 2, 1, 0, 4)
    ).reshape(2 * D // P, P, H // P * P)
    # w1pair[q, p, (j*4 + kk)*128 + c] = w1r tile j's k-tile 4q+kk
    # (j = mp*2+gu, mp<2): the startup stationaries packed contiguously
    # in 4-k pieces so their DMAs move 4 KB per partition (full ring
    # rate) and the first piece + first x half opens the stream gate.
    w1pair = np.ascontiguousarray(
        w1r[:4].reshape(4, P, 4, 4, P).transpose(2, 1, 0, 3, 4)
    ).reshape(4, P, 4 * 4 * P)
    # w2r[p, k2, h] = W2[k2*128+p, h]
    w2bf = w2_e.astype(NPBF)
    w2r = np.ascontiguousarray(w2bf.reshape(D // P, P, H).transpose(1, 0, 2))
    return {"xt": xt, "w1": w1r, "w1pair": w1pair, "w2": w2r}


def run_sharded(hidden_states, gate_up_proj, down_proj, trace=False, **kwargs):
    """Run on 8 cores; returns (full_output, BassKernelResults)."""
    hidden_states = np.asarray(hidden_states, dtype=np.float32)
    gate_up_proj = np.asarray(gate_up_proj, dtype=np.float32)
    down_proj = np.asarray(down_proj, dtype=np.float32)

    nc = _get_nc()
    in_maps = [
        _prep_core(
            (
                hidden_states[e * T_CORE : (e + 1) * T_CORE],
                gate_up_proj[e],
                down_proj[e],
            )
        )
        for e in range(NCORES)
    ]
    res = run_bass_kernel_spmd(
        nc, in_maps, core_ids=list(range(NCORES)), trace=trace, **kwargs
    )
    out = np.concatenate(
        [res.results[e]["out"].astype(np.float32) for e in range(NCORES)], axis=0
    )
    return out, res


def kernel(hidden_states, gate_up_proj, down_proj):
    import os

    # The NTFF trace path needs antenv.axon_hooks, absent in this image;
    # make sure a stray BASS_TRACE env can't route us into it.
    os.environ["BASS_NEVER_TRACE"] = "1"
    try:
        # NOTE: do NOT run the NEFF twice for DVFS warm-up, however
        # tempting (cold device = 1.61 ms vs warm 1.345 ms): a second
        # execution produces a second per-core NTFF and
        # gauge.profiler.convert_ntffs_to_json ASSERTS on multiple
        # ntffs mapping to one json path — it would crash the profiling
        # harness entirely.
        out, _ = run_sharded(hidden_states, gate_up_proj, down_proj)
    finally:
        del os.environ["BASS_NEVER_TRACE"]
    return out
